# revision 13
# baseline (speedup 1.0000x reference)
"""Trainium2 Bass kernel for nn_CustomLoss_82343112999105.

Loss: per-sample select(match, MSE(coords), CE(classes)) averaged over batch.
  predictions: [B, 114] f32  (cols 0:2 coords, 2:114 = 112 class logits)
  targets:     [B, 2]  f32
  out: scalar f32 = mean over B of (match ? mse : ce)
    tc       = floor((tx-22)/.25)*8 + floor((ty-120)/.25)
    match    = (argmax(logits) == tc)
    mse      = mean((coords - targets)^2)   (over the 2 coords)
    ce       = logsumexp(logits) - logits[tc]  (logits are O(5): no max-shift)

Sharding: pure data-parallel over 8 NeuronCores; each core computes a partial
sum over its 131072 rows; host adds the 8 partials and divides by B.

Per-core plan (memory-bound target ~165us = 57MB / 358GB/s):
  rows are processed in chunks of T*128; row = chunk*T*128 + t*128 + p lives
  on partition p as sub-tile t.  Per chunk:
  - 2 DMAs: predictions [128, T*114], targets [128, T*2]
  - ScalarE: one batched Exp over all T sub-tiles (3D AP), ~100us total
  - VectorE per sub-tile t:
      gather:  scalar_tensor_tensor (iota==tc)*logits, accum -> logits[tc]
               (3-operand form runs 1x: ~117ns)
      countgt: tensor_scalar (logits > logits[tc]), accum -> count
               (plain tensor_scalar runs 2x for f32 SBUF: ~58ns)
      match  = (count == 0)  — exact (ties are measure-zero, see note)
  - sumexp per sub-tile: tensor_scalar bypass+accum over exp; runs on GpSimd
    for most chunks to offload VectorE (both engines land ~200us).
  - small [128,T] batched ops: tc from targets (exact floor via +2^23
    round-then-correct; mod is not valid ISA), ce/match/mse/select.
Final: per-chunk row sums -> [128, NCH] -> [128,1] -> GpSimd partition
all-reduce -> DMA one f32 out.

Match-exactness note: count_gt==0 differs from argmax==tc only when the row
max is attained by tc AND an earlier class with the bit-identical f32 value
(P ~ 2e-9 per row) — negligible.
"""

import sys
from contextlib import ExitStack

import numpy as np

sys.path.insert(0, "/opt/trn_rl_repo")

import concourse.bass as bass  # noqa: E402
import concourse.tile as tile  # noqa: E402
from concourse import bacc, bass_isa, mybir  # noqa: E402
from concourse.bass_utils import run_bass_kernel_spmd  # noqa: E402

F32 = mybir.dt.float32
F16 = mybir.dt.float16
ALU = mybir.AluOpType
ACTF = mybir.ActivationFunctionType
AXIS = mybir.AxisListType

B = 1048576
W = 114          # columns in predictions
C = 112          # number of classes
NCORES = 8
P = 128
RPC = B // NCORES  # rows per core
T_SUB = 64
# fraction of chunks whose rowmax runs on GpSimd instead of VectorE
POOL_ROWMAX_NUM = 0
POOL_ROWMAX_DEN = 16


def build_program(rpc=RPC, t_sub=T_SUB, num_devices=NCORES):
    """Build the per-core Bass/Tile program. Same program runs SPMD on all cores."""
    nch = rpc // (P * t_sub)
    assert nch * P * t_sub == rpc

    nc = bacc.Bacc("TRN2", target_bir_lowering=False, debug=False,
                   num_devices=num_devices)

    pred_d = nc.dram_tensor("predictions", [rpc, W], F32, kind="ExternalInput").ap()
    tgt_d = nc.dram_tensor("targets", [rpc, 2], F32, kind="ExternalInput").ap()
    out_d = nc.dram_tensor("out", [1, 1], F32, kind="ExternalOutput").ap()

    # DRAM views: row = ((n*t_sub)+t)*128 + p
    pred_v = pred_d.rearrange("(n t p) w -> n p t w", t=t_sub, p=P)
    tgt_v = tgt_d.rearrange("(n t p) x -> n p t x", t=t_sub, p=P)

    with tile.TileContext(nc) as tc, ExitStack() as ctx:
        singles = ctx.enter_context(tc.tile_pool(name="singles", bufs=1))
        pred_pool = ctx.enter_context(tc.tile_pool(name="pred", bufs=2))
        exp_pool = ctx.enter_context(tc.tile_pool(name="exp", bufs=2))
        tgt_pool = ctx.enter_context(tc.tile_pool(name="tgt", bufs=2))
        stat_pool = ctx.enter_context(tc.tile_pool(name="stat", bufs=2))
        junk_pool = ctx.enter_context(tc.tile_pool(name="junk", bufs=2))

        # constants
        iota_c = singles.tile([P, C], F32)
        nc.gpsimd.iota(iota_c, pattern=[[1, C]], base=0, channel_multiplier=0,
                       allow_small_or_imprecise_dtypes=True)
        chunk_tots = singles.tile([P, nch], F32)

        for n in range(nch):
            pool_rowmax = (n * POOL_ROWMAX_DEN) // nch < POOL_ROWMAX_NUM

            pred = pred_pool.tile([P, t_sub * W], F32, tag="pred")
            pred3 = pred.rearrange("p (t w) -> p t w", w=W)
            nc.sync.dma_start(out=pred3, in_=pred_v[n])

            tgt = tgt_pool.tile([P, t_sub * 2], F32, tag="tgt")
            tgt3 = tgt.rearrange("p (t x) -> p t x", x=2)
            nc.sync.dma_start(out=tgt3, in_=tgt_v[n])

            logits3 = pred3[:, :, 2:W]          # [P, t_sub, 112] strided
            coords3 = pred3[:, :, 0:2]          # [P, t_sub, 2] strided

            # ---- target class tc = 8*floor((tx-22)*4) + floor((ty-120)*4)
            tx = tgt3[:, :, 0]
            ty = tgt3[:, :, 1]
            # floor(s) for s in [0,16): r = round-to-nearest(s) via the +2^23
            # trick (mod is not valid tensor_scalar ISA), then r - (r > s).
            TWO23 = 8388608.0

            def floor_of(src, kind):
                s = stat_pool.tile([P, t_sub], F32, tag=f"s{kind}")
                r = stat_pool.tile([P, t_sub], F32, tag=f"r{kind}")
                fix = stat_pool.tile([P, t_sub], F32, tag=f"x{kind}")
                f = stat_pool.tile([P, t_sub], F32, tag=f"f{kind}")
                off = 22.0 if kind == "x" else 120.0
                nc.vector.tensor_scalar(out=s, in0=src, scalar1=off,
                                        scalar2=4.0, op0=ALU.subtract,
                                        op1=ALU.mult)
                nc.vector.tensor_scalar(out=r, in0=s, scalar1=TWO23,
                                        scalar2=TWO23, op0=ALU.add,
                                        op1=ALU.subtract)
                nc.vector.scalar_tensor_tensor(out=fix, in0=r, scalar=0.0,
                                               in1=s, op0=ALU.add,
                                               op1=ALU.is_gt)
                nc.vector.scalar_tensor_tensor(out=f, in0=r, scalar=0.0,
                                               in1=fix, op0=ALU.add,
                                               op1=ALU.subtract)
                return f

            fx = floor_of(tx, "x")
            fy = floor_of(ty, "y")
            tcl = stat_pool.tile([P, t_sub], F32, tag="tcl")
            nc.vector.scalar_tensor_tensor(out=tcl, in0=fx, scalar=8.0, in1=fy,
                                           op0=ALU.mult, op1=ALU.add)

            # ---- gather logits[tc] per sub-tile: accum((iota==tc)*logits)
            tgt_l = stat_pool.tile([P, t_sub], F32, tag="tgt_l")
            junk = junk_pool.tile([P, C], F32, tag="junk")
            for t in range(t_sub):
                nc.vector.scalar_tensor_tensor(
                    out=junk, in0=iota_c, scalar=tcl[:, t:t + 1],
                    in1=logits3[:, t, :], op0=ALU.is_equal, op1=ALU.mult,
                    accum_out=tgt_l[:, t:t + 1])

            # ---- exp of all logits (one batched ACT instruction), fp16 out:
            # fp16 keeps sumexp accurate to ~1e-7 on the final mean and lets
            # the DVE sumexp pass run in 4x mode (2-byte packed SBUF).
            expc = exp_pool.tile([P, t_sub * C], F16, tag="expc")
            exp3 = expc.rearrange("p (t c) -> p t c", c=C)
            nc.scalar.activation(out=exp3, in_=logits3, func=ACTF.Exp)

            # ---- sumexp per sub-tile: tensor_scalar +accum (4x fp16 on DVE)
            sumexp = stat_pool.tile([P, t_sub], F32, tag="sumexp")
            junk_s = junk_pool.tile([P, C], F16, tag="junk_s")
            for t in range(t_sub):
                nc.vector.tensor_scalar(
                    out=junk_s, in0=exp3[:, t, :], scalar1=0.0,
                    scalar2=None, op0=ALU.add, op1=ALU.add,
                    accum_out=sumexp[:, t:t + 1])

            # ---- row max of logits (exact f32). GpSimd's native top-8 max
            # keeps the DVE free for the gather (Pool is otherwise idle);
            # TensorScalarPtr is not legal on Pool, InstMax is.
            if pool_rowmax:
                rmax8 = stat_pool.tile([P, t_sub * 8], F32, tag="rmax8")
                rmax3 = rmax8.rearrange("p (t k) -> p t k", k=8)
                for t in range(t_sub):
                    nc.gpsimd.max(rmax3[:, t, :], logits3[:, t, :])
                rmax_col = rmax3[:, :, 0]       # [P, t_sub] stride 8
            else:
                rmax = stat_pool.tile([P, t_sub], F32, tag="rmax")
                junk2 = junk_pool.tile([P, C], F32, tag="junk2")
                for t in range(t_sub):
                    nc.vector.tensor_scalar(
                        out=junk2, in0=logits3[:, t, :], scalar1=0.0,
                        scalar2=None, op0=ALU.add, op1=ALU.max,
                        accum_out=rmax[:, t:t + 1])
                rmax_col = rmax[:, :]

            # ---- phase 2 (batched [P, t_sub] ops)
            lnse = stat_pool.tile([P, t_sub], F32, tag="lnse")
            nc.scalar.activation(out=lnse, in_=sumexp, func=ACTF.Ln)
            ce = stat_pool.tile([P, t_sub], F32, tag="ce")
            nc.vector.scalar_tensor_tensor(out=ce, in0=lnse, scalar=0.0,
                                           in1=tgt_l, op0=ALU.add,
                                           op1=ALU.subtract)
            mbit = stat_pool.tile([P, t_sub], F32, tag="mbit")
            nc.vector.scalar_tensor_tensor(out=mbit, in0=tgt_l, scalar=0.0,
                                           in1=rmax_col, op0=ALU.add,
                                           op1=ALU.is_equal)
            d = stat_pool.tile([P, t_sub * 2], F32, tag="d")
            d3 = d.rearrange("p (t x) -> p t x", x=2)
            nc.vector.scalar_tensor_tensor(out=d3, in0=coords3, scalar=0.0,
                                           in1=tgt3, op0=ALU.add,
                                           op1=ALU.subtract)
            sq = stat_pool.tile([P, t_sub * 2], F32, tag="sq")
            sq3 = sq.rearrange("p (t x) -> p t x", x=2)
            nc.scalar.activation(out=sq, in_=d, func=ACTF.Square)
            mse2 = stat_pool.tile([P, t_sub], F32, tag="mse2")
            nc.vector.scalar_tensor_tensor(out=mse2, in0=sq3[:, :, 0],
                                           scalar=0.0, in1=sq3[:, :, 1],
                                           op0=ALU.add, op1=ALU.add)
            # diff = 0.5*mse2 - ce ; per = ce + mbit*diff
            diff = stat_pool.tile([P, t_sub], F32, tag="diff")
            nc.vector.scalar_tensor_tensor(out=diff, in0=mse2, scalar=0.5,
                                           in1=ce, op0=ALU.mult,
                                           op1=ALU.subtract)
            prod = stat_pool.tile([P, t_sub], F32, tag="prod")
            nc.vector.scalar_tensor_tensor(out=prod, in0=mbit, scalar=1.0,
                                           in1=diff, op0=ALU.mult,
                                           op1=ALU.mult)
            per = stat_pool.tile([P, t_sub], F32, tag="per")
            nc.vector.scalar_tensor_tensor(out=per, in0=ce, scalar=0.0,
                                           in1=prod, op0=ALU.add, op1=ALU.add)
            nc.vector.tensor_reduce(out=chunk_tots[:, n:n + 1], in_=per,
                                    axis=AXIS.X, op=ALU.add)

        # ---- finale: total over chunks, then over partitions
        core_tot = singles.tile([P, 1], F32)
        nc.vector.tensor_reduce(out=core_tot, in_=chunk_tots, axis=AXIS.X,
                                op=ALU.add)
        allred = singles.tile([P, 1], F32)
        nc.gpsimd.partition_all_reduce(out_ap=allred, in_ap=core_tot,
                                       channels=P,
                                       reduce_op=bass_isa.ReduceOp.add)
        nc.sync.dma_start(out=out_d, in_=allred[0:1, :])

    nc.compile()
    return nc


_CACHE = {}


def _get_program():
    key = (RPC, T_SUB)
    if key not in _CACHE:
        _CACHE[key] = build_program(rpc=RPC, t_sub=T_SUB)
    return _CACHE[key]


def kernel(predictions, targets, **_):
    predictions = np.ascontiguousarray(np.asarray(predictions, dtype=np.float32))
    targets = np.ascontiguousarray(np.asarray(targets, dtype=np.float32))
    assert predictions.shape == (B, W) and targets.shape == (B, 2)

    nc = _get_program()
    in_maps = []
    for i in range(NCORES):
        sl = slice(i * RPC, (i + 1) * RPC)
        in_maps.append({"predictions": predictions[sl], "targets": targets[sl]})
    res = run_bass_kernel_spmd(nc, in_maps, core_ids=list(range(NCORES)))
    total = sum(float(res.results[i]["out"][0, 0]) for i in range(NCORES))
    return np.float32(total / B)


# revision 17
# speedup vs baseline: 1.0089x; 1.0089x over previous
"""Trainium2 Bass kernel for nn_CustomLoss_82343112999105.

Loss: per-sample select(match, MSE(coords), CE(classes)) averaged over batch.
  predictions: [B, 114] f32  (cols 0:2 coords, 2:114 = 112 class logits)
  targets:     [B, 2]  f32
  out: scalar f32 = mean over B of (match ? mse : ce)
    tc       = floor((tx-22)/.25)*8 + floor((ty-120)/.25)
    match    = (argmax(logits) == tc)
    mse      = mean((coords - targets)^2)   (over the 2 coords)
    ce       = logsumexp(logits) - logits[tc]  (logits are O(5): no max-shift)

Sharding: pure data-parallel over 8 NeuronCores; each core computes a partial
sum over its 131072 rows; host adds the 8 partials and divides by B.

Per-core plan (memory-bound target ~165us = 57MB / 358GB/s):
  rows are processed in chunks of T*128; row = chunk*T*128 + t*128 + p lives
  on partition p as sub-tile t.  Per chunk:
  - 2 DMAs: predictions [128, T*114], targets [128, T*2]
  - ScalarE: one batched Exp over all T sub-tiles (3D AP), ~100us total
  - VectorE per sub-tile t (accum_out is per-partition-per-instruction, so
    these are forced to one instruction per 128-row group):
      gather:  scalar_tensor_tensor (iota==tc)*logits, accum -> logits[tc]
               (3-operand form runs 1x: ~177ns incl. SBUF access)
      rowmax:  tensor_scalar max-accumulate over logits (2x f32: ~118ns)
      sumexp:  tensor_scalar add-accumulate over fp16 exp (4x: ~89ns)
      match  = (logits[tc] == rowmax), exact in f32.
  - small [128,T] batched ops: tc from targets (exact floor via +2^23
    round-then-correct; mod is not valid ISA), ce/match/mse/select.
Final: per-chunk row sums -> [128, NCH] -> [128,1] -> GpSimd partition
all-reduce -> DMA one f32 out.

Engine notes learned the hard way: this walrus build rejects ALL generic
tensor ops (TensorScalarPtr, TensorTensor) on the Pool/GpSimd engine, so
the three per-row reductions cannot be offloaded from the VectorE; PE only
reduces across partitions (useless here since the per-row gather pins rows
to partitions); ACT per-instruction overhead (~190ns + 187ns accum-read)
rules out per-sub-tile ACT accumulation. VectorE is therefore the
bottleneck at ~400us/core vs the ~165us HBM roofline.
"""

import sys
from contextlib import ExitStack

import numpy as np

sys.path.insert(0, "/opt/trn_rl_repo")

import concourse.bass as bass  # noqa: E402
import concourse.tile as tile  # noqa: E402
from concourse import bacc, bass_isa, mybir  # noqa: E402
from concourse.bass_utils import run_bass_kernel_spmd  # noqa: E402

F32 = mybir.dt.float32
F16 = mybir.dt.float16
ALU = mybir.AluOpType
ACTF = mybir.ActivationFunctionType
AXIS = mybir.AxisListType

B = 1048576
W = 114          # columns in predictions
C = 112          # number of classes
NCORES = 8
P = 128
RPC = B // NCORES  # rows per core
T_SUB = 64
# fraction of chunks whose rowmax runs on GpSimd instead of VectorE
POOL_ROWMAX_NUM = 0
POOL_ROWMAX_DEN = 16


def build_program(rpc=RPC, t_sub=T_SUB, num_devices=NCORES):
    """Build the per-core Bass/Tile program. Same program runs SPMD on all cores."""
    nch = rpc // (P * t_sub)
    assert nch * P * t_sub == rpc

    nc = bacc.Bacc("TRN2", target_bir_lowering=False, debug=False,
                   num_devices=num_devices)

    pred_d = nc.dram_tensor("predictions", [rpc, W], F32, kind="ExternalInput").ap()
    tgt_d = nc.dram_tensor("targets", [rpc, 2], F32, kind="ExternalInput").ap()
    out_d = nc.dram_tensor("out", [1, 1], F32, kind="ExternalOutput").ap()

    # DRAM views: row = ((n*t_sub)+t)*128 + p
    pred_v = pred_d.rearrange("(n t p) w -> n p t w", t=t_sub, p=P)
    tgt_v = tgt_d.rearrange("(n t p) x -> n p t x", t=t_sub, p=P)

    with tile.TileContext(nc) as tc, ExitStack() as ctx:
        singles = ctx.enter_context(tc.tile_pool(name="singles", bufs=1))
        pred_pool = ctx.enter_context(tc.tile_pool(name="pred", bufs=2))
        exp_pool = ctx.enter_context(tc.tile_pool(name="exp", bufs=2))
        tgt_pool = ctx.enter_context(tc.tile_pool(name="tgt", bufs=2))
        stat_pool = ctx.enter_context(tc.tile_pool(name="stat", bufs=2))
        junk_pool = ctx.enter_context(tc.tile_pool(name="junk", bufs=2))

        # constants
        iota_c = singles.tile([P, C], F32)
        nc.gpsimd.iota(iota_c, pattern=[[1, C]], base=0, channel_multiplier=0,
                       allow_small_or_imprecise_dtypes=True)
        chunk_tots = singles.tile([P, nch], F32)

        for n in range(nch):
            pool_rowmax = (n * POOL_ROWMAX_DEN) // nch < POOL_ROWMAX_NUM

            pred = pred_pool.tile([P, t_sub * W], F32, tag="pred")
            pred3 = pred.rearrange("p (t w) -> p t w", w=W)
            nc.sync.dma_start(out=pred3, in_=pred_v[n])

            tgt = tgt_pool.tile([P, t_sub * 2], F32, tag="tgt")
            tgt3 = tgt.rearrange("p (t x) -> p t x", x=2)
            nc.sync.dma_start(out=tgt3, in_=tgt_v[n])

            logits3 = pred3[:, :, 2:W]          # [P, t_sub, 112] strided
            coords3 = pred3[:, :, 0:2]          # [P, t_sub, 2] strided

            # ---- target class tc = 8*floor((tx-22)*4) + floor((ty-120)*4)
            tx = tgt3[:, :, 0]
            ty = tgt3[:, :, 1]
            # floor(s) for s in [0,16): r = round-to-nearest(s) via the +2^23
            # trick (mod is not valid tensor_scalar ISA), then r - (r > s).
            TWO23 = 8388608.0

            def floor_of(src, kind):
                s = stat_pool.tile([P, t_sub], F32, tag=f"s{kind}")
                r = stat_pool.tile([P, t_sub], F32, tag=f"r{kind}")
                fix = stat_pool.tile([P, t_sub], F32, tag=f"x{kind}")
                f = stat_pool.tile([P, t_sub], F32, tag=f"f{kind}")
                off = 22.0 if kind == "x" else 120.0
                nc.vector.tensor_scalar(out=s, in0=src, scalar1=off,
                                        scalar2=4.0, op0=ALU.subtract,
                                        op1=ALU.mult)
                nc.vector.tensor_scalar(out=r, in0=s, scalar1=TWO23,
                                        scalar2=TWO23, op0=ALU.add,
                                        op1=ALU.subtract)
                nc.vector.scalar_tensor_tensor(out=fix, in0=r, scalar=0.0,
                                               in1=s, op0=ALU.add,
                                               op1=ALU.is_gt)
                nc.vector.scalar_tensor_tensor(out=f, in0=r, scalar=0.0,
                                               in1=fix, op0=ALU.add,
                                               op1=ALU.subtract)
                return f

            fx = floor_of(tx, "x")
            fy = floor_of(ty, "y")
            tcl = stat_pool.tile([P, t_sub], F32, tag="tcl")
            nc.vector.scalar_tensor_tensor(out=tcl, in0=fx, scalar=8.0, in1=fy,
                                           op0=ALU.mult, op1=ALU.add)

            # ---- gather logits[tc] per sub-tile: accum((iota==tc)*logits)
            tgt_l = stat_pool.tile([P, t_sub], F32, tag="tgt_l")
            junk = junk_pool.tile([P, C], F32, tag="junk")
            for t in range(t_sub):
                nc.vector.scalar_tensor_tensor(
                    out=junk, in0=iota_c, scalar=tcl[:, t:t + 1],
                    in1=logits3[:, t, :], op0=ALU.is_equal, op1=ALU.mult,
                    accum_out=tgt_l[:, t:t + 1])

            # ---- exp of all logits (one batched ACT instruction), fp16 out:
            # fp16 keeps sumexp accurate to ~1e-7 on the final mean and lets
            # the DVE sumexp pass run in 4x mode (2-byte packed SBUF).
            expc = exp_pool.tile([P, t_sub * C], F16, tag="expc")
            exp3 = expc.rearrange("p (t c) -> p t c", c=C)
            nc.scalar.activation(out=exp3, in_=logits3, func=ACTF.Exp)

            # ---- sumexp per sub-tile: tensor_scalar +accum (4x fp16 on DVE)
            sumexp = stat_pool.tile([P, t_sub], F32, tag="sumexp")
            junk_s = junk_pool.tile([P, C], F16, tag="junk_s")
            for t in range(t_sub):
                nc.vector.tensor_scalar(
                    out=junk_s, in0=exp3[:, t, :], scalar1=0.0,
                    scalar2=None, op0=ALU.add, op1=ALU.add,
                    accum_out=sumexp[:, t:t + 1])

            # ---- row max of logits (exact f32), per sub-tile via
            # tensor_scalar max-accumulate (2x f32 SBUF mode on DVE).
            # Note: this walrus build rejects TensorScalarPtr/TensorTensor
            # on the Pool engine, so the max cannot be offloaded to GpSimd.
            if pool_rowmax:
                cur, width, lvl = logits3, C, 0
                while width > 1:
                    half = (width + 1) // 2   # overlap by 1 is fine for max
                    nxt = stat_pool.tile([P, t_sub * half], F32,
                                         tag=f"mx{lvl}")
                    nxt3 = nxt.rearrange("p (t k) -> p t k", k=half)
                    nc.gpsimd.tensor_tensor(out=nxt3, in0=cur[:, :, 0:half],
                                            in1=cur[:, :, width - half:width],
                                            op=ALU.max)
                    cur, width, lvl = nxt3, half, lvl + 1
                rmax_col = cur[:, :, 0]         # [P, t_sub]
            else:
                rmax = stat_pool.tile([P, t_sub], F32, tag="rmax")
                junk2 = junk_pool.tile([P, C], F32, tag="junk2")
                for t in range(t_sub):
                    nc.vector.tensor_scalar(
                        out=junk2, in0=logits3[:, t, :], scalar1=0.0,
                        scalar2=None, op0=ALU.add, op1=ALU.max,
                        accum_out=rmax[:, t:t + 1])
                rmax_col = rmax[:, :]

            # ---- phase 2 (batched [P, t_sub] ops)
            lnse = stat_pool.tile([P, t_sub], F32, tag="lnse")
            nc.scalar.activation(out=lnse, in_=sumexp, func=ACTF.Ln)
            ce = stat_pool.tile([P, t_sub], F32, tag="ce")
            nc.vector.scalar_tensor_tensor(out=ce, in0=lnse, scalar=0.0,
                                           in1=tgt_l, op0=ALU.add,
                                           op1=ALU.subtract)
            mbit = stat_pool.tile([P, t_sub], F32, tag="mbit")
            nc.vector.scalar_tensor_tensor(out=mbit, in0=tgt_l, scalar=0.0,
                                           in1=rmax_col, op0=ALU.add,
                                           op1=ALU.is_equal)
            d = stat_pool.tile([P, t_sub * 2], F32, tag="d")
            d3 = d.rearrange("p (t x) -> p t x", x=2)
            nc.vector.scalar_tensor_tensor(out=d3, in0=coords3, scalar=0.0,
                                           in1=tgt3, op0=ALU.add,
                                           op1=ALU.subtract)
            sq = stat_pool.tile([P, t_sub * 2], F32, tag="sq")
            sq3 = sq.rearrange("p (t x) -> p t x", x=2)
            nc.scalar.activation(out=sq, in_=d, func=ACTF.Square)
            mse2 = stat_pool.tile([P, t_sub], F32, tag="mse2")
            nc.vector.scalar_tensor_tensor(out=mse2, in0=sq3[:, :, 0],
                                           scalar=0.0, in1=sq3[:, :, 1],
                                           op0=ALU.add, op1=ALU.add)
            # diff = 0.5*mse2 - ce ; per = ce + mbit*diff
            diff = stat_pool.tile([P, t_sub], F32, tag="diff")
            nc.vector.scalar_tensor_tensor(out=diff, in0=mse2, scalar=0.5,
                                           in1=ce, op0=ALU.mult,
                                           op1=ALU.subtract)
            prod = stat_pool.tile([P, t_sub], F32, tag="prod")
            nc.vector.scalar_tensor_tensor(out=prod, in0=mbit, scalar=1.0,
                                           in1=diff, op0=ALU.mult,
                                           op1=ALU.mult)
            per = stat_pool.tile([P, t_sub], F32, tag="per")
            nc.vector.scalar_tensor_tensor(out=per, in0=ce, scalar=0.0,
                                           in1=prod, op0=ALU.add, op1=ALU.add)
            nc.vector.tensor_reduce(out=chunk_tots[:, n:n + 1], in_=per,
                                    axis=AXIS.X, op=ALU.add)

        # ---- finale: total over chunks, then over partitions
        core_tot = singles.tile([P, 1], F32)
        nc.vector.tensor_reduce(out=core_tot, in_=chunk_tots, axis=AXIS.X,
                                op=ALU.add)
        allred = singles.tile([P, 1], F32)
        nc.gpsimd.partition_all_reduce(out_ap=allred, in_ap=core_tot,
                                       channels=P,
                                       reduce_op=bass_isa.ReduceOp.add)
        nc.sync.dma_start(out=out_d, in_=allred[0:1, :])

    nc.compile()
    return nc


_CACHE = {}


def _get_program():
    key = (RPC, T_SUB)
    if key not in _CACHE:
        _CACHE[key] = build_program(rpc=RPC, t_sub=T_SUB)
    return _CACHE[key]


def kernel(predictions, targets, **_):
    predictions = np.ascontiguousarray(np.asarray(predictions, dtype=np.float32))
    targets = np.ascontiguousarray(np.asarray(targets, dtype=np.float32))
    assert predictions.shape == (B, W) and targets.shape == (B, 2)

    nc = _get_program()
    in_maps = []
    for i in range(NCORES):
        sl = slice(i * RPC, (i + 1) * RPC)
        in_maps.append({"predictions": predictions[sl], "targets": targets[sl]})
    res = run_bass_kernel_spmd(nc, in_maps, core_ids=list(range(NCORES)))
    total = sum(float(res.results[i]["out"][0, 0]) for i in range(NCORES))
    return np.float32(total / B)
